# revision 49
# baseline (speedup 1.0000x reference)
"""Trainium2 Bass kernel for nn_CrossAttentionFusion (cross-attention + BitLinear FFN).

Sharding: 8 cores = 4 batches x 2 sequence-halves. Each core:
  - owns 1024 query tokens (sem shard, feature-major),
  - computes K/V for its batch's full 2048 tokens from pro (feature-major),
  - runs full attention for its queries + BitLinear FFN, writes its out^T shard.
No collectives; host does layout prep, weight ternarization and the gather.

v2: fp8 DoubleRow matmuls for all GEMMs except QK^T scores; PE-based softmax
denominator (DR all-ones stationary gives a broadcast denominator directly);
act-quant implemented as a direct fp8 cast with all static scales folded into
the snake/evac constants; bf16 residual trunk; 2-chunk pipeline overlapping
scalar-bound attention with PE-bound FFN2.
"""
import math
import numpy as np
from contextlib import ExitStack

import concourse.bass as bass
import concourse.tile as tile
from concourse import bacc, mybir
from concourse.bass_utils import run_bass_kernel_spmd

F32 = mybir.dt.float32
BF16 = mybir.dt.bfloat16
FP8 = mybir.dt.float8e4
AF = mybir.ActivationFunctionType
ALU = mybir.AluOpType
DR = mybir.MatmulPerfMode.DoubleRow

B, S, DS, DP, H = 4, 2048, 1024, 512, 8
DF = 4 * DS
HD = DS // H          # 128
TOK = 1024            # query tokens per core
N_CORES = 8
EPS = 1e-6
QK_SCALE = 1.0 / math.sqrt(HD)
WSC = 64.0            # host premultiplier on Wq/Wk/Wv/Wo before fp8 cast

P = 128
M_SEM = DS // P       # 8
M_PRO = DP // P       # 4
M_FF = DF // P        # 32
NT_Q = TOK // 512     # 2
NT_K = S // P         # 16


def build_nc(debug_outs=False):
    nc = bacc.Bacc("TRN2", target_bir_lowering=False, debug=False,
                   num_devices=N_CORES)

    semT = nc.dram_tensor("semT", [P, M_SEM, TOK], F32, kind="ExternalInput").ap()
    proT = nc.dram_tensor("proT", [P, M_PRO, S], F32, kind="ExternalInput").ap()
    wq_d = nc.dram_tensor("wq", [P, M_SEM, DS], FP8, kind="ExternalInput").ap()
    wk_d = nc.dram_tensor("wk", [P, M_PRO, DS], FP8, kind="ExternalInput").ap()
    wv_d = nc.dram_tensor("wv", [P, M_PRO, DS], FP8, kind="ExternalInput").ap()
    wo_d = nc.dram_tensor("wo", [P, M_SEM, DS], FP8, kind="ExternalInput").ap()
    w1_d = nc.dram_tensor("w1q", [P, M_SEM, DF], FP8, kind="ExternalInput").ap()
    w2_d = nc.dram_tensor("w2q", [P, M_FF, DS], FP8, kind="ExternalInput").ap()
    gsem = nc.dram_tensor("gsem", [P, M_SEM], F32, kind="ExternalInput").ap()
    gpro = nc.dram_tensor("gpro", [P, M_PRO], F32, kind="ExternalInput").ap()
    gff = nc.dram_tensor("gff", [P, M_SEM], F32, kind="ExternalInput").ap()
    bq_d = nc.dram_tensor("bq", [P, M_SEM], F32, kind="ExternalInput").ap()
    bk_d = nc.dram_tensor("bk", [P, M_SEM], F32, kind="ExternalInput").ap()
    boe_d = nc.dram_tensor("boe", [P, M_SEM], F32, kind="ExternalInput").ap()
    alp_d = nc.dram_tensor("alphap", [P, M_FF], F32, kind="ExternalInput").ap()
    rbp_d = nc.dram_tensor("rbp", [P, M_FF], F32, kind="ExternalInput").ap()
    c2_d = nc.dram_tensor("c2", [P, 1], F32, kind="ExternalInput").ap()
    outT = nc.dram_tensor("outT", [DS, TOK], F32, kind="ExternalOutput").ap()

    dbg = {}
    if debug_outs:
        for name, shape, dt in [
            ("dbg_semn", [P, M_SEM, TOK], FP8), ("dbg_q", [P, M_SEM, TOK], FP8),
            ("dbg_k", [P, M_SEM, S], FP8), ("dbg_v", [P, NT_K, DS], FP8),
            ("dbg_ctx", [P, M_SEM, TOK], FP8),
            ("dbg_semout", [P, M_SEM, TOK], BF16),
            ("dbg_xq", [P, M_SEM, TOK], FP8), ("dbg_hq", [P, M_FF, TOK], FP8),
        ]:
            dbg[name] = nc.dram_tensor(name, shape, dt, kind="ExternalOutput").ap()

    with tile.TileContext(nc) as tc, ExitStack() as top:
        persist = top.enter_context(tc.tile_pool(name="persist", bufs=1))
        # PSUM: 2x2 banks for scores + 2x2 banks for everything else
        ps_big = top.enter_context(tc.tile_pool(name="ps_big", bufs=2, space="PSUM"))
        ps_mm = top.enter_context(tc.tile_pool(name="ps_mm", bufs=2, space="PSUM"))

        # ---- constants ----
        ones_bf = persist.tile([P, 1], BF16)
        nc.vector.memset(ones_bf[:], 1.0)
        ones_f32 = persist.tile([1, P], F32)
        nc.vector.memset(ones_f32[:], 1.0)
        ones_dr = persist.tile([P, 2, P], FP8)
        nc.vector.memset(ones_dr[:].rearrange("p a b -> p (a b)"), 1.0)
        eps_t = persist.tile([P, 1], F32)
        nc.vector.memset(eps_t[:], EPS)

        gsem_sb = persist.tile([P, M_SEM], F32)
        gpro_sb = persist.tile([P, M_PRO], F32)
        gff_sb = persist.tile([P, M_SEM], F32)
        bq_sb = persist.tile([P, M_SEM], F32)
        bk_sb = persist.tile([P, M_SEM], F32)
        boe_sb = persist.tile([P, M_SEM], F32)
        alp_sb = persist.tile([P, M_FF], F32)
        rbp_sb = persist.tile([P, M_FF], F32)
        c2_sb = persist.tile([P, 1], F32)
        for ap_d, t in [(gsem, gsem_sb), (gpro, gpro_sb), (gff, gff_sb),
                        (bq_d, bq_sb), (bk_d, bk_sb), (boe_d, boe_sb),
                        (alp_d, alp_sb), (rbp_d, rbp_sb), (c2_d, c2_sb)]:
            nc.sync.dma_start(t[:], ap_d[:])

        # ---- big weights, resident; DMA'd later on the gpsimd queue ----
        wf = top.enter_context(tc.tile_pool(name="wf", bufs=1, side="right"))
        w1_sb = wf.tile([P, M_SEM, DF], FP8)
        w2_sb = wf.tile([P, M_FF, DS], FP8)

        def rmsnorm(pool, xs, nm, T, g_sb, out_fp8, Dtot, sq_vec=False):
            """feature-major rmsnorm -> fp8. xs(m) -> [P, T] f32/bf16 tile.
            Fully chunk-pipelined: each 512-column chunk runs square -> sum ->
            ln -> exp -> broadcast -> scale independently."""
            for ch in range(T // 512):
                ps = ps_mm.tile([P, 1024], F32, tag="mm")
                for m in range(nm):
                    sq = pool.tile([P, 512], BF16, tag="nsq", bufs=3)
                    xsl = xs(m)[:, ch * 512:(ch + 1) * 512]
                    if sq_vec:
                        nc.vector.tensor_tensor(sq[:], xsl, xsl, op=ALU.mult)
                    else:
                        nc.scalar.activation(sq[:], xsl, AF.Square)
                    nc.tensor.matmul(ps[0:1, 0:512], ones_bf[:], sq[:],
                                     start=(m == 0), stop=(m == nm - 1))
                rs_row = pool.tile([1, 512], F32, tag="rsrow", bufs=2)
                nc.scalar.activation(rs_row[:], ps[0:1, 0:512], AF.Ln,
                                     bias=eps_t[0:1, :], scale=1.0 / Dtot)
                nc.scalar.activation(rs_row[:], rs_row[:], AF.Exp, scale=-0.5)
                psb = ps_mm.tile([P, 1024], F32, tag="mm")
                nc.tensor.matmul(psb[:, 0:512], ones_f32[:], rs_row[:],
                                 start=True, stop=True)
                for m in range(nm):
                    nc.vector.scalar_tensor_tensor(
                        out=out_fp8[:, m, ch * 512:(ch + 1) * 512],
                        in0=xs(m)[:, ch * 512:(ch + 1) * 512],
                        scalar=g_sb[:, m:m + 1], in1=psb[:, 0:512],
                        op0=ALU.mult, op1=ALU.mult)

        # out-proj weights: allocated early so pool release order stays LIFO
        es_wo = ExitStack()
        pwo = es_wo.enter_context(tc.tile_pool(name="pwo", bufs=1))
        wo_sb = pwo.tile([P, M_SEM, DS], FP8)

        # QKV weights allocated before pin so their DMAs start immediately
        es_w1 = ExitStack()
        pw1 = es_w1.enter_context(tc.tile_pool(name="pw1", bufs=1))
        wq_sb = pw1.tile([P, M_SEM, DS], FP8)
        wk_sb = pw1.tile([P, M_PRO, DS], FP8)
        wv_sb = pw1.tile([P, M_PRO, DS], FP8)

        # ================= P0: input norms =================
        es_nrm = ExitStack()
        pnorm = es_nrm.enter_context(tc.tile_pool(name="pnorm", bufs=1))
        semn = pnorm.tile([P, M_SEM, TOK], FP8)
        pron = pnorm.tile([P, M_PRO, S], FP8)

        with tc.tile_pool(name="pin", bufs=1) as pin:
            semT_sb = pin.tile([P, M_SEM, TOK], F32)
            proT_sb = pin.tile([P, M_PRO, S], F32)
            # chunked so the first rmsnorm square can start after 0.5 MB;
            # spread across three DMA queues so transfers run in parallel
            for m in range(M_SEM):
                nc.sync.dma_start(semT_sb[:, m, :], semT[:, m, :])
            for m in range(M_PRO):
                nc.gpsimd.dma_start(proT_sb[:, m, :], proT[:, m, :])
            nc.sync.dma_start(wq_sb[:], wq_d[:])
            nc.gpsimd.dma_start(wk_sb[:], wk_d[:])
            nc.gpsimd.dma_start(wv_sb[:], wv_d[:])
            nc.scalar.dma_start(w1_sb[:], w1_d[:])
            nc.scalar.dma_start(w2_sb[:], w2_d[:])
            nc.scalar.dma_start(wo_sb[:], wo_d[:])
            rmsnorm(pin, lambda m: semT_sb[:, m, :], M_SEM, TOK, gsem_sb,
                    semn, DS)
            rmsnorm(pin, lambda m: proT_sb[:, m, :], M_PRO, S, gpro_sb,
                    pron, DP)

        if debug_outs:
            nc.sync.dma_start(dbg["dbg_semn"][:], semn[:])

        # ================= P1: Q/K/V projections =================
        # pff (FFN scratch) sits below pqkv on the right stack so q/k/v can
        # be released before the FFN tail while pff lives on
        es_ff = ExitStack()
        pff = es_ff.enter_context(tc.tile_pool(name="pff", bufs=1,
                                               side="right"))
        es_qkv = ExitStack()
        pqkv = es_qkv.enter_context(tc.tile_pool(name="pqkv", bufs=1,
                                                 side="right"))
        q_sb = pqkv.tile([P, M_SEM, TOK], FP8)
        k_sb = pqkv.tile([P, M_SEM, S], FP8)
        v_sb = pqkv.tile([P, NT_K, DS], FP8)

        for m in range(M_SEM):
            ps = (ps_mm if m % 2 else ps_big).tile(
                [P, 1024], F32, tag="mm" if m % 2 else "big")
            for n in range(NT_Q):
                for kp in range(M_SEM // 2):
                    nc.tensor.matmul(
                        ps[:, n * 512:(n + 1) * 512],
                        wq_sb[:, 2 * kp:2 * kp + 2, m * P:(m + 1) * P],
                        semn[:, 2 * kp:2 * kp + 2, n * 512:(n + 1) * 512],
                        start=(kp == 0), stop=(kp == M_SEM // 2 - 1),
                        perf_mode=DR)
            nc.vector.tensor_scalar(q_sb[:, m, :], ps[:], 1.0 / WSC,
                                    bq_sb[:, m:m + 1], ALU.mult, ALU.add)
        for m in range(M_SEM):
            for chp in range(2):
                ps = (ps_mm if chp else ps_big).tile(
                    [P, 1024], F32, tag="mm" if chp else "big")
                for half in range(2):
                    ch = 2 * chp + half
                    for kp in range(M_PRO // 2):
                        nc.tensor.matmul(
                            ps[:, half * 512:(half + 1) * 512],
                            wk_sb[:, 2 * kp:2 * kp + 2, m * P:(m + 1) * P],
                            pron[:, 2 * kp:2 * kp + 2,
                                 ch * 512:(ch + 1) * 512],
                            start=(kp == 0), stop=(kp == M_PRO // 2 - 1),
                            perf_mode=DR)
                nc.vector.tensor_scalar(
                    k_sb[:, m, chp * 1024:(chp + 1) * 1024], ps[:],
                    1.0 / WSC, bk_sb[:, m:m + 1], ALU.mult, ALU.add)
        for mt in range(NT_K):
            ps = (ps_mm if mt % 2 else ps_big).tile(
                [P, 1024], F32, tag="mm" if mt % 2 else "big")
            for ch in range(2):
                for kp in range(M_PRO // 2):
                    nc.tensor.matmul(
                        ps[:, ch * 512:(ch + 1) * 512],
                        pron[:, 2 * kp:2 * kp + 2, mt * P:(mt + 1) * P],
                        wv_sb[:, 2 * kp:2 * kp + 2, ch * 512:(ch + 1) * 512],
                        start=(kp == 0), stop=(kp == M_PRO // 2 - 1),
                        perf_mode=DR)
            nc.vector.tensor_scalar(v_sb[:, mt, :], ps[:], 1.0 / WSC, None,
                                    ALU.mult)
        es_nrm.close()
        es_w1.close()

        if debug_outs:
            nc.sync.dma_start(dbg["dbg_q"][:], q_sb[:])
            nc.sync.dma_start(dbg["dbg_k"][:], k_sb[:])
            nc.sync.dma_start(dbg["dbg_v"][:], v_sb[:])

        # ============ pipeline state tiles ============
        es_pipe = ExitStack()
        ppipe = es_pipe.enter_context(tc.tile_pool(name="ppipe", bufs=1))
        semout_n = [ppipe.tile([P, M_SEM, 512], BF16, name=f"so{n}")
                    for n in range(NT_Q)]
        xq_n = [ppipe.tile([P, M_SEM, 512], FP8, name=f"xq{n}")
                for n in range(NT_Q)]
        es_attn = ExitStack()
        pattn = es_attn.enter_context(tc.tile_pool(name="pattn", bufs=1))

        deferred = []

        def drain(k):
            for _ in range(k):
                if deferred:
                    deferred.pop(0)()

        def attn_chunk(n, ctx_t):
            """softmax(q_n K^T) V -> ctx_t [P, M_SEM, 512] fp8 (head-major)."""
            for h in range(H):
                pt = pattn.tile([P, NT_K, 512], FP8, tag="pt", bufs=2)
                for g in range(8):
                    ps = ps_big.tile([P, 1024], F32, tag="big")
                    for q2 in range(2):
                        mt = 2 * g + q2
                        nc.tensor.matmul(
                            ps[:, q2 * 512:(q2 + 1) * 512],
                            k_sb[:, h, mt * P:(mt + 1) * P],
                            q_sb[:, h, n * 512:(n + 1) * 512],
                            start=True, stop=True)
                    drain(1)
                    nc.scalar.activation(pt[:, 2 * g:2 * g + 2, :], ps[:],
                                         AF.Exp, scale=QK_SCALE)

                def fin(h=h, pt=pt):
                    pdc = ps_mm.tile([P, 1024], F32, tag="mm")
                    for j in range(8):
                        nc.tensor.matmul(pdc[:, 0:512], ones_dr[:],
                                         pt[:, 2 * j:2 * j + 2, :],
                                         start=(j == 0), stop=(j == 7),
                                         perf_mode=DR)
                    rden = pattn.tile([P, 512], F32, tag="rden", bufs=2)
                    nc.vector.reciprocal_approx_fast(rden[:], pdc[:, 0:512])
                    for j in range(8):
                        nc.tensor.matmul(pdc[:, 512:1024],
                                         v_sb[:, 2 * j:2 * j + 2,
                                              h * P:(h + 1) * P],
                                         pt[:, 2 * j:2 * j + 2, :],
                                         start=(j == 0), stop=(j == 7),
                                         perf_mode=DR)
                    nc.vector.tensor_tensor(ctx_t[:, h, :], pdc[:, 512:1024],
                                            rden[:], op=ALU.mult)
                deferred.append(fin)
            drain(len(deferred))

        def out_proj(n, ctx_t):
            for mg in range(M_SEM // 2):
                semres = ppipe.tile([P, 2, 512], F32, tag="semres", bufs=2)
                nc.sync.dma_start(semres[:],
                                  semT[:, 2 * mg:2 * mg + 2,
                                       n * 512:(n + 1) * 512])
                ps = ps_mm.tile([P, 1024], F32, tag="mm")
                for half in range(2):
                    m = 2 * mg + half
                    for kp in range(M_SEM // 2):
                        nc.tensor.matmul(
                            ps[:, half * 512:(half + 1) * 512],
                            wo_sb[:, 2 * kp:2 * kp + 2, m * P:(m + 1) * P],
                            ctx_t[:, 2 * kp:2 * kp + 2, :],
                            start=(kp == 0), stop=(kp == M_SEM // 2 - 1),
                            perf_mode=DR)
                for half in range(2):
                    m = 2 * mg + half
                    t = pff.tile([P, 512], BF16, tag="oproj", bufs=3)
                    nc.vector.tensor_scalar(t[:],
                                            ps[:, half * 512:(half + 1) * 512],
                                            1.0 / WSC, boe_sb[:, m:m + 1],
                                            ALU.mult, ALU.add)
                    nc.vector.tensor_tensor(semout_n[n][:, m, :], t[:],
                                            semres[:, half, :], op=ALU.add)

        def ff_norm(n):
            rmsnorm(pff, lambda m: semout_n[n][:, m, :], M_SEM, 512, gff_sb,
                    xq_n[n], DS, sq_vec=True)

        def ffn1(n, hq_t, zip_fn=()):
            zip_fn = list(zip_fn)
            for mg in range(M_FF // 2):
                if mg % 4 == 1 and zip_fn:
                    zip_fn.pop(0)()
                ps = (ps_mm if mg % 2 else ps_big).tile(
                    [P, 1024], F32, tag="mm" if mg % 2 else "big")
                for half in range(2):
                    m = 2 * mg + half
                    for kp in range(M_SEM // 2):
                        nc.tensor.matmul(
                            ps[:, half * 512:(half + 1) * 512],
                            w1_sb[:, 2 * kp:2 * kp + 2, m * P:(m + 1) * P],
                            xq_n[n][:, 2 * kp:2 * kp + 2, :],
                            start=(kp == 0), stop=(kp == M_SEM // 2 - 1),
                            perf_mode=DR)
                sn = pff.tile([P, 1024], BF16, tag="sn", bufs=2)
                for half in range(2):
                    m = 2 * mg + half
                    nc.scalar.activation(sn[:, half * 512:(half + 1) * 512],
                                         ps[:, half * 512:(half + 1) * 512],
                                         AF.Sin, scale=alp_sb[:, m:m + 1])
                sq = pff.tile([P, 1024], BF16, tag="sqf", bufs=2)
                nc.vector.tensor_tensor(sq[:], sn[:], sn[:], op=ALU.mult)
                for half in range(2):
                    m = 2 * mg + half
                    nc.vector.scalar_tensor_tensor(
                        out=hq_t[:, m, :],
                        in0=sq[:, half * 512:(half + 1) * 512],
                        scalar=rbp_sb[:, m:m + 1],
                        in1=ps[:, half * 512:(half + 1) * 512],
                        op0=ALU.mult, op1=ALU.add)

        def ffn2_tile(n, mg, hq_t):
            ps = ps_mm.tile([P, 1024], F32, tag="mm")
            for half in range(2):
                m = 2 * mg + half
                for kp in range(M_FF // 2):
                    nc.tensor.matmul(
                        ps[:, half * 512:(half + 1) * 512],
                        w2_sb[:, 2 * kp:2 * kp + 2, m * P:(m + 1) * P],
                        hq_t[:, 2 * kp:2 * kp + 2, :],
                        start=(kp == 0), stop=(kp == M_FF // 2 - 1),
                        perf_mode=DR)
            for half in range(2):
                m = 2 * mg + half
                yo = pff.tile([P, 512], F32, tag="yo", bufs=2)
                nc.vector.scalar_tensor_tensor(
                    out=yo[:], in0=ps[:, half * 512:(half + 1) * 512],
                    scalar=c2_sb[:, 0:1], in1=semout_n[n][:, m, :],
                    op0=ALU.mult, op1=ALU.add)
                nc.sync.dma_start(outT[m * P:(m + 1) * P,
                                       n * 512:(n + 1) * 512], yo[:])

        # ================= P2..P6: pipeline =================
        # attn(0) -> attn(1) zipped with [out_proj(0), ff_norm(0)] ->
        # out_proj(1), ff_norm(1) -> FFN1(0) -> FFN1(1) zipped with FFN2(0)
        # -> FFN2(1).  All Sin activations are grouped at the tail so the
        # scalar engine switches act tables once (exp/ln family -> sin).
        ctx0 = pattn.tile([P, M_SEM, 512], FP8, tag="ctxt", bufs=1, name="ctx0")
        attn_chunk(0, ctx0)

        deferred.append(lambda: out_proj(0, ctx0))
        deferred.append(lambda: ff_norm(0))
        ctx1 = pattn.tile([P, M_SEM, 512], FP8, tag="ctxt", bufs=1, name="ctx1")
        attn_chunk(1, ctx1)
        out_proj(1, ctx1)
        ff_norm(1)
        if debug_outs:
            nc.sync.dma_start(dbg["dbg_ctx"][:, :, 0:512], ctx0[:])
            nc.sync.dma_start(dbg["dbg_semout"][:, :, 0:512], semout_n[0][:])
            nc.sync.dma_start(dbg["dbg_xq"][:, :, 0:512], xq_n[0][:])
            nc.sync.dma_start(dbg["dbg_ctx"][:, :, 512:1024], ctx1[:])
            nc.sync.dma_start(dbg["dbg_semout"][:, :, 512:1024], semout_n[1][:])
            nc.sync.dma_start(dbg["dbg_xq"][:, :, 512:1024], xq_n[1][:])

        es_attn.close()   # pt/rden/ctx freed
        es_qkv.close()    # q/k/v freed before the FFN tail
        es_hq = ExitStack()
        phq = es_hq.enter_context(tc.tile_pool(name="phq", bufs=2,
                                               side="right"))

        hq0 = phq.tile([P, M_FF, 512], FP8, tag="hq", bufs=2, name="hq0")
        ffn1(0, hq0)
        if debug_outs:
            nc.sync.dma_start(dbg["dbg_hq"][:, :, 0:512], hq0[:])

        hq1 = phq.tile([P, M_FF, 512], FP8, tag="hq", bufs=2, name="hq1")
        ffn1(1, hq1, zip_fn=[lambda mg=mg: ffn2_tile(0, mg, hq0)
                             for mg in range(M_SEM // 2)])
        if debug_outs:
            nc.sync.dma_start(dbg["dbg_hq"][:, :, 512:1024], hq1[:])
        for mg in range(M_SEM // 2):
            ffn2_tile(1, mg, hq1)

        es_hq.close()
        es_pipe.close()
        es_ff.close()
        es_wo.close()

    nc.compile()
    return nc


_NC_CACHE = {}


def _get_nc(debug_outs=False):
    key = bool(debug_outs)
    if key not in _NC_CACHE:
        _NC_CACHE[key] = build_nc(debug_outs)
    return _NC_CACHE[key]


def _feat_major(x, nm):
    """[rows, cols] -> [128, nm, cols] with rows = nm*128 split (m p) -> p m."""
    rows, cols = x.shape
    return np.ascontiguousarray(
        x.reshape(nm, P, cols).transpose(1, 0, 2))


def make_in_maps(inputs):
    """Host-side shard + layout prep. inputs: dict of full np arrays."""
    import ml_dtypes
    f8 = ml_dtypes.float8_e4m3fn
    f32 = np.float32
    sem = np.asarray(inputs["sem"], f32)
    pro = np.asarray(inputs["pro"], f32)

    def cols(v, nm):
        return np.ascontiguousarray(np.asarray(v, f32).reshape(nm, P).T)

    W1 = np.asarray(inputs["W1"], f32)
    W2 = np.asarray(inputs["W2"], f32)
    s1 = 1.0 / max(np.abs(W1).mean(dtype=np.float64), 1e-5)
    s2 = 1.0 / max(np.abs(W2).mean(dtype=np.float64), 1e-5)
    w1t = np.clip(np.round(W1 * s1), -1, 1).astype(f32)   # [DF, DS] ternary
    w2t = np.clip(np.round(W2 * s2), -1, 1).astype(f32)   # [DS, DF] ternary

    Wo = np.asarray(inputs["Wo"], f32)
    boe = (np.asarray(inputs["bo"], f32)
           + Wo @ np.asarray(inputs["bv"], f32))

    alpha = np.asarray(inputs["alpha"], f32)
    beta = np.asarray(inputs["beta"], f32)
    alphap = (alpha / s1).astype(f32)
    rbp = (s1 / (beta + 1e-9)).astype(f32)
    c2 = np.full((P, 1), 1.0 / (s1 * s2), f32)

    common = {
        "gsem": cols(inputs["g_sem"], M_SEM),
        "gpro": cols(inputs["g_pro"], M_PRO),
        "gff": cols(inputs["g_ff"], M_SEM),
        "bq": cols(inputs["bq"], M_SEM),
        "bk": cols(inputs["bk"], M_SEM),
        "boe": cols(boe, M_SEM),
        "alphap": cols(alphap, M_FF),
        "rbp": cols(rbp, M_FF),
        "c2": c2,
        "wq": _feat_major(np.asarray(inputs["Wq"], f32).T * WSC, M_SEM).astype(f8),
        "wk": _feat_major(np.asarray(inputs["Wk"], f32).T * WSC, M_PRO).astype(f8),
        "wv": _feat_major(np.asarray(inputs["Wv"], f32).T * WSC, M_PRO).astype(f8),
        "wo": _feat_major(Wo.T * WSC, M_SEM).astype(f8),
        "w1q": _feat_major(np.ascontiguousarray(w1t.T), M_SEM).astype(f8),
        "w2q": _feat_major(np.ascontiguousarray(w2t.T), M_FF).astype(f8),
    }

    in_maps = []
    for c in range(N_CORES):
        b, half = c // 2, c % 2
        m = dict(common)
        m["semT"] = _feat_major(
            np.ascontiguousarray(sem[b, half * TOK:(half + 1) * TOK, :].T),
            M_SEM)
        m["proT"] = _feat_major(np.ascontiguousarray(pro[b].T), M_PRO)
        in_maps.append(m)
    return in_maps


def assemble_out(results):
    out = np.empty((B, S, DS), np.float32)
    for c in range(N_CORES):
        b, half = c // 2, c % 2
        out[b, half * TOK:(half + 1) * TOK, :] = results[c]["outT"].T
    return out


def kernel(**inputs):
    nc = _get_nc()
    in_maps = make_in_maps(inputs)
    res = run_bass_kernel_spmd(nc, in_maps, core_ids=list(range(N_CORES)))
    return assemble_out(res.results)


# revision 53
# speedup vs baseline: 1.0899x; 1.0899x over previous
"""Trainium2 Bass kernel for nn_CrossAttentionFusion (cross-attention + BitLinear FFN).

Sharding: 8 cores = 4 batches x 2 sequence-halves. Each core:
  - owns 1024 query tokens (sem shard, feature-major),
  - computes K/V for its batch's full 2048 tokens from pro (feature-major),
  - runs full attention for its queries + BitLinear FFN, writes its out^T shard.
No collectives; host does layout prep, weight ternarization and the gather.

v2: fp8 DoubleRow matmuls for all GEMMs except QK^T scores; PE-based softmax
denominator (DR all-ones stationary gives a broadcast denominator directly);
act-quant implemented as a direct fp8 cast with all static scales folded into
the snake/evac constants; bf16 residual trunk; 2-chunk pipeline overlapping
scalar-bound attention with PE-bound FFN2.
"""
import math
import numpy as np
from contextlib import ExitStack

import concourse.bass as bass
import concourse.tile as tile
from concourse import bacc, mybir
from concourse.bass_utils import run_bass_kernel_spmd

F32 = mybir.dt.float32
BF16 = mybir.dt.bfloat16
FP8 = mybir.dt.float8e4
AF = mybir.ActivationFunctionType
ALU = mybir.AluOpType
DR = mybir.MatmulPerfMode.DoubleRow

B, S, DS, DP, H = 4, 2048, 1024, 512, 8
DF = 4 * DS
HD = DS // H          # 128
TOK = 1024            # query tokens per core
N_CORES = 8
EPS = 1e-6
QK_SCALE = 1.0 / math.sqrt(HD)
WSC = 64.0            # host premultiplier on Wq/Wk/Wv/Wo before fp8 cast

P = 128
M_SEM = DS // P       # 8
M_PRO = DP // P       # 4
M_FF = DF // P        # 32
NT_Q = TOK // 512     # 2
NT_K = S // P         # 16


def build_nc(debug_outs=False):
    nc = bacc.Bacc("TRN2", target_bir_lowering=False, debug=False,
                   num_devices=N_CORES)

    semT = nc.dram_tensor("semT", [P, M_SEM, TOK], F32, kind="ExternalInput").ap()
    proT = nc.dram_tensor("proT", [P, M_PRO, S], F32, kind="ExternalInput").ap()
    wq_d = nc.dram_tensor("wq", [P, M_SEM, DS], FP8, kind="ExternalInput").ap()
    wk_d = nc.dram_tensor("wk", [P, M_PRO, DS], FP8, kind="ExternalInput").ap()
    wv_d = nc.dram_tensor("wv", [P, M_PRO, DS], FP8, kind="ExternalInput").ap()
    wo_d = nc.dram_tensor("wo", [P, M_SEM, DS], FP8, kind="ExternalInput").ap()
    w1_d = nc.dram_tensor("w1q", [P, M_SEM, DF], FP8, kind="ExternalInput").ap()
    w2_d = nc.dram_tensor("w2q", [P, M_FF, DS], FP8, kind="ExternalInput").ap()
    gsem = nc.dram_tensor("gsem", [P, M_SEM], F32, kind="ExternalInput").ap()
    gpro = nc.dram_tensor("gpro", [P, M_PRO], F32, kind="ExternalInput").ap()
    gff = nc.dram_tensor("gff", [P, M_SEM], F32, kind="ExternalInput").ap()
    bq_d = nc.dram_tensor("bq", [P, M_SEM], F32, kind="ExternalInput").ap()
    bk_d = nc.dram_tensor("bk", [P, M_SEM], F32, kind="ExternalInput").ap()
    boe_d = nc.dram_tensor("boe", [P, M_SEM], F32, kind="ExternalInput").ap()
    alp_d = nc.dram_tensor("alphap", [P, M_FF], F32, kind="ExternalInput").ap()
    rbp_d = nc.dram_tensor("rbp", [P, M_FF], F32, kind="ExternalInput").ap()
    c2_d = nc.dram_tensor("c2", [P, 1], F32, kind="ExternalInput").ap()
    outT = nc.dram_tensor("outT", [DS, TOK], F32, kind="ExternalOutput").ap()

    dbg = {}
    if debug_outs:
        for name, shape, dt in [
            ("dbg_semn", [P, M_SEM, TOK], FP8), ("dbg_q", [P, M_SEM, TOK], FP8),
            ("dbg_k", [P, M_SEM, S], FP8), ("dbg_v", [P, NT_K, DS], FP8),
            ("dbg_ctx", [P, M_SEM, TOK], FP8),
            ("dbg_semout", [P, M_SEM, TOK], BF16),
            ("dbg_xq", [P, M_SEM, TOK], FP8), ("dbg_hq", [P, M_FF, TOK], FP8),
        ]:
            dbg[name] = nc.dram_tensor(name, shape, dt, kind="ExternalOutput").ap()

    with tile.TileContext(nc) as tc, ExitStack() as top:
        persist = top.enter_context(tc.tile_pool(name="persist", bufs=1))
        # PSUM: 2x2 banks for scores + 2x2 banks for everything else
        ps_big = top.enter_context(tc.tile_pool(name="ps_big", bufs=2, space="PSUM"))
        ps_mm = top.enter_context(tc.tile_pool(name="ps_mm", bufs=2, space="PSUM"))

        # ---- constants ----
        ones_bf = persist.tile([P, 1], BF16)
        nc.vector.memset(ones_bf[:], 1.0)
        ones_f32 = persist.tile([1, P], F32)
        nc.vector.memset(ones_f32[:], 1.0)
        ones_dr = persist.tile([P, 2, P], FP8)
        nc.vector.memset(ones_dr[:].rearrange("p a b -> p (a b)"), 1.0)
        eps_t = persist.tile([P, 1], F32)
        nc.vector.memset(eps_t[:], EPS)

        gsem_sb = persist.tile([P, M_SEM], F32)
        gpro_sb = persist.tile([P, M_PRO], F32)
        gff_sb = persist.tile([P, M_SEM], F32)
        bq_sb = persist.tile([P, M_SEM], F32)
        bk_sb = persist.tile([P, M_SEM], F32)
        boe_sb = persist.tile([P, M_SEM], F32)
        alp_sb = persist.tile([P, M_FF], F32)
        rbp_sb = persist.tile([P, M_FF], F32)
        c2_sb = persist.tile([P, 1], F32)
        for ap_d, t in [(gsem, gsem_sb), (gpro, gpro_sb), (gff, gff_sb),
                        (bq_d, bq_sb), (bk_d, bk_sb), (boe_d, boe_sb),
                        (alp_d, alp_sb), (rbp_d, rbp_sb), (c2_d, c2_sb)]:
            nc.sync.dma_start(t[:], ap_d[:])

        # ---- big weights, resident; DMA'd later on the gpsimd queue ----
        wf = top.enter_context(tc.tile_pool(name="wf", bufs=1, side="right"))
        w1_sb = wf.tile([P, M_SEM, DF], FP8)
        w2_sb = wf.tile([P, M_FF, DS], FP8)

        def rmsnorm(pool, xs, nm, T, g_sb, out_fp8, Dtot, sq_vec=False):
            """feature-major rmsnorm -> fp8. xs(m) -> [P, T] f32/bf16 tile.
            Fully chunk-pipelined: each 512-column chunk runs square -> sum ->
            ln -> exp -> broadcast -> scale independently."""
            for ch in range(T // 512):
                ps = ps_mm.tile([P, 1024], F32, tag="mm")
                for m in range(nm):
                    sq = pool.tile([P, 512], BF16, tag="nsq", bufs=3)
                    xsl = xs(m)[:, ch * 512:(ch + 1) * 512]
                    if sq_vec:
                        nc.vector.tensor_tensor(sq[:], xsl, xsl, op=ALU.mult)
                    else:
                        nc.scalar.activation(sq[:], xsl, AF.Square)
                    nc.tensor.matmul(ps[0:1, 0:512], ones_bf[:], sq[:],
                                     start=(m == 0), stop=(m == nm - 1))
                rs_row = pool.tile([1, 512], F32, tag="rsrow", bufs=2)
                nc.scalar.activation(rs_row[:], ps[0:1, 0:512], AF.Ln,
                                     bias=eps_t[0:1, :], scale=1.0 / Dtot)
                nc.scalar.activation(rs_row[:], rs_row[:], AF.Exp, scale=-0.5)
                psb = ps_mm.tile([P, 1024], F32, tag="mm")
                nc.tensor.matmul(psb[:, 0:512], ones_f32[:], rs_row[:],
                                 start=True, stop=True)
                for m in range(nm):
                    nc.vector.scalar_tensor_tensor(
                        out=out_fp8[:, m, ch * 512:(ch + 1) * 512],
                        in0=xs(m)[:, ch * 512:(ch + 1) * 512],
                        scalar=g_sb[:, m:m + 1], in1=psb[:, 0:512],
                        op0=ALU.mult, op1=ALU.mult)

        # out-proj weights: allocated early so pool release order stays LIFO
        es_wo = ExitStack()
        pwo = es_wo.enter_context(tc.tile_pool(name="pwo", bufs=1))
        wo_sb = pwo.tile([P, M_SEM, DS], FP8)

        # QKV weights allocated before pin so their DMAs start immediately
        es_w1 = ExitStack()
        pw1 = es_w1.enter_context(tc.tile_pool(name="pw1", bufs=1))
        wq_sb = pw1.tile([P, M_SEM, DS], FP8)
        wk_sb = pw1.tile([P, M_PRO, DS], FP8)
        wv_sb = pw1.tile([P, M_PRO, DS], FP8)

        # ================= P0: input norms =================
        es_nrm = ExitStack()
        pnorm = es_nrm.enter_context(tc.tile_pool(name="pnorm", bufs=1))
        semn = pnorm.tile([P, M_SEM, TOK], FP8)
        pron = pnorm.tile([P, M_PRO, S], FP8)

        with tc.tile_pool(name="pin", bufs=1) as pin:
            semT_sb = pin.tile([P, M_SEM, TOK], F32)
            proT_sb = pin.tile([P, M_PRO, S], F32)
            # chunked so the first rmsnorm square can start after 0.5 MB;
            # spread across three DMA queues so transfers run in parallel
            for m in range(M_SEM):
                nc.sync.dma_start(semT_sb[:, m, :], semT[:, m, :])
            for m in range(M_PRO):
                nc.sync.dma_start(proT_sb[:, m, :], proT[:, m, :])
            nc.sync.dma_start(wq_sb[:], wq_d[:])
            nc.sync.dma_start(wk_sb[:], wk_d[:])
            nc.sync.dma_start(wv_sb[:], wv_d[:])
            nc.sync.dma_start(w1_sb[:], w1_d[:])
            nc.sync.dma_start(w2_sb[:], w2_d[:])
            nc.sync.dma_start(wo_sb[:], wo_d[:])
            rmsnorm(pin, lambda m: semT_sb[:, m, :], M_SEM, TOK, gsem_sb,
                    semn, DS)
            rmsnorm(pin, lambda m: proT_sb[:, m, :], M_PRO, S, gpro_sb,
                    pron, DP)

        if debug_outs:
            nc.sync.dma_start(dbg["dbg_semn"][:], semn[:])

        # ================= P1: Q/K/V projections =================
        # pff (FFN scratch) sits below pqkv on the right stack so q/k/v can
        # be released before the FFN tail while pff lives on
        es_ff = ExitStack()
        pff = es_ff.enter_context(tc.tile_pool(name="pff", bufs=1,
                                               side="right"))
        es_qkv = ExitStack()
        pqkv = es_qkv.enter_context(tc.tile_pool(name="pqkv", bufs=1,
                                                 side="right"))
        q_sb = pqkv.tile([P, M_SEM, TOK], FP8)
        k_sb = pqkv.tile([P, M_SEM, S], FP8)
        v_sb = pqkv.tile([P, NT_K, DS], FP8)

        for m in range(M_SEM):
            ps = (ps_mm if m % 2 else ps_big).tile(
                [P, 1024], F32, tag="mm" if m % 2 else "big")
            for n in range(NT_Q):
                for kp in range(M_SEM // 2):
                    nc.tensor.matmul(
                        ps[:, n * 512:(n + 1) * 512],
                        wq_sb[:, 2 * kp:2 * kp + 2, m * P:(m + 1) * P],
                        semn[:, 2 * kp:2 * kp + 2, n * 512:(n + 1) * 512],
                        start=(kp == 0), stop=(kp == M_SEM // 2 - 1),
                        perf_mode=DR)
            nc.vector.tensor_scalar(q_sb[:, m, :], ps[:], 1.0 / WSC,
                                    bq_sb[:, m:m + 1], ALU.mult, ALU.add)
        for m in range(M_SEM):
            for chp in range(2):
                ps = (ps_mm if chp else ps_big).tile(
                    [P, 1024], F32, tag="mm" if chp else "big")
                for half in range(2):
                    ch = 2 * chp + half
                    for kp in range(M_PRO // 2):
                        nc.tensor.matmul(
                            ps[:, half * 512:(half + 1) * 512],
                            wk_sb[:, 2 * kp:2 * kp + 2, m * P:(m + 1) * P],
                            pron[:, 2 * kp:2 * kp + 2,
                                 ch * 512:(ch + 1) * 512],
                            start=(kp == 0), stop=(kp == M_PRO // 2 - 1),
                            perf_mode=DR)
                nc.vector.tensor_scalar(
                    k_sb[:, m, chp * 1024:(chp + 1) * 1024], ps[:],
                    1.0 / WSC, bk_sb[:, m:m + 1], ALU.mult, ALU.add)
        for mt in range(NT_K):
            ps = (ps_mm if mt % 2 else ps_big).tile(
                [P, 1024], F32, tag="mm" if mt % 2 else "big")
            for ch in range(2):
                for kp in range(M_PRO // 2):
                    nc.tensor.matmul(
                        ps[:, ch * 512:(ch + 1) * 512],
                        pron[:, 2 * kp:2 * kp + 2, mt * P:(mt + 1) * P],
                        wv_sb[:, 2 * kp:2 * kp + 2, ch * 512:(ch + 1) * 512],
                        start=(kp == 0), stop=(kp == M_PRO // 2 - 1),
                        perf_mode=DR)
            nc.vector.tensor_scalar(v_sb[:, mt, :], ps[:], 1.0 / WSC, None,
                                    ALU.mult)
        es_nrm.close()
        es_w1.close()

        if debug_outs:
            nc.sync.dma_start(dbg["dbg_q"][:], q_sb[:])
            nc.sync.dma_start(dbg["dbg_k"][:], k_sb[:])
            nc.sync.dma_start(dbg["dbg_v"][:], v_sb[:])

        # ============ pipeline state tiles ============
        es_pipe = ExitStack()
        ppipe = es_pipe.enter_context(tc.tile_pool(name="ppipe", bufs=1))
        semout_n = [ppipe.tile([P, M_SEM, 512], BF16, name=f"so{n}")
                    for n in range(NT_Q)]
        xq_n = [ppipe.tile([P, M_SEM, 512], FP8, name=f"xq{n}")
                for n in range(NT_Q)]
        es_attn = ExitStack()
        pattn = es_attn.enter_context(tc.tile_pool(name="pattn", bufs=1))

        deferred = []

        def drain(k):
            for _ in range(k):
                if deferred:
                    deferred.pop(0)()

        def attn_chunk(n, ctx_t):
            """softmax(q_n K^T) V -> ctx_t [P, M_SEM, 512] fp8 (head-major)."""
            for h in range(H):
                pt = pattn.tile([P, NT_K, 512], FP8, tag="pt", bufs=2)
                for g in range(8):
                    ps = ps_big.tile([P, 1024], F32, tag="big")
                    for q2 in range(2):
                        mt = 2 * g + q2
                        nc.tensor.matmul(
                            ps[:, q2 * 512:(q2 + 1) * 512],
                            k_sb[:, h, mt * P:(mt + 1) * P],
                            q_sb[:, h, n * 512:(n + 1) * 512],
                            start=True, stop=True)
                    drain(1)
                    nc.scalar.activation(pt[:, 2 * g:2 * g + 2, :], ps[:],
                                         AF.Exp, scale=QK_SCALE)

                box = []

                def fin_den(h=h, pt=pt, box=box):
                    pdc = ps_mm.tile([P, 1024], F32, tag="mm")
                    for j in range(8):
                        nc.tensor.matmul(pdc[:, 0:512], ones_dr[:],
                                         pt[:, 2 * j:2 * j + 2, :],
                                         start=(j == 0), stop=(j == 7),
                                         perf_mode=DR)
                    rden = pattn.tile([P, 512], F32, tag="rden", bufs=2)
                    nc.vector.reciprocal_approx_fast(rden[:], pdc[:, 0:512])
                    box.append((pdc, rden))

                def fin_ctx(h=h, pt=pt, box=box):
                    pdc, rden = box.pop()
                    for j in range(8):
                        nc.tensor.matmul(pdc[:, 512:1024],
                                         v_sb[:, 2 * j:2 * j + 2,
                                              h * P:(h + 1) * P],
                                         pt[:, 2 * j:2 * j + 2, :],
                                         start=(j == 0), stop=(j == 7),
                                         perf_mode=DR)
                    nc.vector.tensor_tensor(ctx_t[:, h, :], pdc[:, 512:1024],
                                            rden[:], op=ALU.mult)
                deferred.append(fin_den)
                deferred.append(fin_ctx)
            drain(len(deferred))

        def out_proj(n, ctx_t, mgs=None):
            for mg in (range(M_SEM // 2) if mgs is None else mgs):
                semres = ppipe.tile([P, 2, 512], F32, tag="semres", bufs=2)
                nc.sync.dma_start(semres[:],
                                  semT[:, 2 * mg:2 * mg + 2,
                                       n * 512:(n + 1) * 512])
                ps = ps_mm.tile([P, 1024], F32, tag="mm")
                for half in range(2):
                    m = 2 * mg + half
                    for kp in range(M_SEM // 2):
                        nc.tensor.matmul(
                            ps[:, half * 512:(half + 1) * 512],
                            wo_sb[:, 2 * kp:2 * kp + 2, m * P:(m + 1) * P],
                            ctx_t[:, 2 * kp:2 * kp + 2, :],
                            start=(kp == 0), stop=(kp == M_SEM // 2 - 1),
                            perf_mode=DR)
                for half in range(2):
                    m = 2 * mg + half
                    t = pff.tile([P, 512], BF16, tag="oproj", bufs=3)
                    nc.vector.tensor_scalar(t[:],
                                            ps[:, half * 512:(half + 1) * 512],
                                            1.0 / WSC, boe_sb[:, m:m + 1],
                                            ALU.mult, ALU.add)
                    nc.vector.tensor_tensor(semout_n[n][:, m, :], t[:],
                                            semres[:, half, :], op=ALU.add)

        def ff_norm(n):
            rmsnorm(pff, lambda m: semout_n[n][:, m, :], M_SEM, 512, gff_sb,
                    xq_n[n], DS, sq_vec=True)

        def ffn1(n, hq_t, zip_fn=()):
            zip_fn = list(zip_fn)
            for mg in range(M_FF // 2):
                if mg % 4 == 1 and zip_fn:
                    zip_fn.pop(0)()
                ps = (ps_mm if mg % 2 else ps_big).tile(
                    [P, 1024], F32, tag="mm" if mg % 2 else "big")
                for half in range(2):
                    m = 2 * mg + half
                    for kp in range(M_SEM // 2):
                        nc.tensor.matmul(
                            ps[:, half * 512:(half + 1) * 512],
                            w1_sb[:, 2 * kp:2 * kp + 2, m * P:(m + 1) * P],
                            xq_n[n][:, 2 * kp:2 * kp + 2, :],
                            start=(kp == 0), stop=(kp == M_SEM // 2 - 1),
                            perf_mode=DR)
                sn = pff.tile([P, 1024], BF16, tag="sn", bufs=2)
                for half in range(2):
                    m = 2 * mg + half
                    nc.scalar.activation(sn[:, half * 512:(half + 1) * 512],
                                         ps[:, half * 512:(half + 1) * 512],
                                         AF.Sin, scale=alp_sb[:, m:m + 1])
                sq = pff.tile([P, 1024], BF16, tag="sqf", bufs=2)
                nc.vector.tensor_tensor(sq[:], sn[:], sn[:], op=ALU.mult)
                for half in range(2):
                    m = 2 * mg + half
                    nc.vector.scalar_tensor_tensor(
                        out=hq_t[:, m, :],
                        in0=sq[:, half * 512:(half + 1) * 512],
                        scalar=rbp_sb[:, m:m + 1],
                        in1=ps[:, half * 512:(half + 1) * 512],
                        op0=ALU.mult, op1=ALU.add)

        def ffn2_tile(n, mg, hq_t):
            ps = ps_mm.tile([P, 1024], F32, tag="mm")
            for half in range(2):
                m = 2 * mg + half
                for kp in range(M_FF // 2):
                    nc.tensor.matmul(
                        ps[:, half * 512:(half + 1) * 512],
                        w2_sb[:, 2 * kp:2 * kp + 2, m * P:(m + 1) * P],
                        hq_t[:, 2 * kp:2 * kp + 2, :],
                        start=(kp == 0), stop=(kp == M_FF // 2 - 1),
                        perf_mode=DR)
            for half in range(2):
                m = 2 * mg + half
                yo = pff.tile([P, 512], F32, tag="yo", bufs=2)
                nc.vector.scalar_tensor_tensor(
                    out=yo[:], in0=ps[:, half * 512:(half + 1) * 512],
                    scalar=c2_sb[:, 0:1], in1=semout_n[n][:, m, :],
                    op0=ALU.mult, op1=ALU.add)
                nc.sync.dma_start(outT[m * P:(m + 1) * P,
                                       n * 512:(n + 1) * 512], yo[:])

        # ================= P2..P6: pipeline =================
        # attn(0) -> attn(1) zipped with [out_proj(0), ff_norm(0)] ->
        # out_proj(1), ff_norm(1) -> FFN1(0) -> FFN1(1) zipped with FFN2(0)
        # -> FFN2(1).  All Sin activations are grouped at the tail so the
        # scalar engine switches act tables once (exp/ln family -> sin).
        ctx0 = pattn.tile([P, M_SEM, 512], FP8, tag="ctxt", bufs=1, name="ctx0")
        attn_chunk(0, ctx0)

        for mg in range(M_SEM // 2):
            deferred.append(lambda mg=mg: out_proj(0, ctx0, mgs=[mg]))
        deferred.append(lambda: ff_norm(0))
        ctx1 = pattn.tile([P, M_SEM, 512], FP8, tag="ctxt", bufs=1, name="ctx1")
        attn_chunk(1, ctx1)
        out_proj(1, ctx1)
        ff_norm(1)
        if debug_outs:
            nc.sync.dma_start(dbg["dbg_ctx"][:, :, 0:512], ctx0[:])
            nc.sync.dma_start(dbg["dbg_semout"][:, :, 0:512], semout_n[0][:])
            nc.sync.dma_start(dbg["dbg_xq"][:, :, 0:512], xq_n[0][:])
            nc.sync.dma_start(dbg["dbg_ctx"][:, :, 512:1024], ctx1[:])
            nc.sync.dma_start(dbg["dbg_semout"][:, :, 512:1024], semout_n[1][:])
            nc.sync.dma_start(dbg["dbg_xq"][:, :, 512:1024], xq_n[1][:])

        es_attn.close()   # pt/rden/ctx freed
        es_qkv.close()    # q/k/v freed before the FFN tail
        es_hq = ExitStack()
        phq = es_hq.enter_context(tc.tile_pool(name="phq", bufs=2,
                                               side="right"))

        hq0 = phq.tile([P, M_FF, 512], FP8, tag="hq", bufs=2, name="hq0")
        ffn1(0, hq0)
        if debug_outs:
            nc.sync.dma_start(dbg["dbg_hq"][:, :, 0:512], hq0[:])

        hq1 = phq.tile([P, M_FF, 512], FP8, tag="hq", bufs=2, name="hq1")
        ffn1(1, hq1, zip_fn=[lambda mg=mg: ffn2_tile(0, mg, hq0)
                             for mg in range(M_SEM // 2)])
        if debug_outs:
            nc.sync.dma_start(dbg["dbg_hq"][:, :, 512:1024], hq1[:])
        for mg in range(M_SEM // 2):
            ffn2_tile(1, mg, hq1)

        es_hq.close()
        es_pipe.close()
        es_ff.close()
        es_wo.close()

    nc.compile()
    return nc


_NC_CACHE = {}


def _get_nc(debug_outs=False):
    key = bool(debug_outs)
    if key not in _NC_CACHE:
        _NC_CACHE[key] = build_nc(debug_outs)
    return _NC_CACHE[key]


def _feat_major(x, nm):
    """[rows, cols] -> [128, nm, cols] with rows = nm*128 split (m p) -> p m."""
    rows, cols = x.shape
    return np.ascontiguousarray(
        x.reshape(nm, P, cols).transpose(1, 0, 2))


def make_in_maps(inputs):
    """Host-side shard + layout prep. inputs: dict of full np arrays."""
    import ml_dtypes
    f8 = ml_dtypes.float8_e4m3fn
    f32 = np.float32
    sem = np.asarray(inputs["sem"], f32)
    pro = np.asarray(inputs["pro"], f32)

    def cols(v, nm):
        return np.ascontiguousarray(np.asarray(v, f32).reshape(nm, P).T)

    W1 = np.asarray(inputs["W1"], f32)
    W2 = np.asarray(inputs["W2"], f32)
    s1 = 1.0 / max(np.abs(W1).mean(dtype=np.float64), 1e-5)
    s2 = 1.0 / max(np.abs(W2).mean(dtype=np.float64), 1e-5)
    w1t = np.clip(np.round(W1 * s1), -1, 1).astype(f32)   # [DF, DS] ternary
    w2t = np.clip(np.round(W2 * s2), -1, 1).astype(f32)   # [DS, DF] ternary

    Wo = np.asarray(inputs["Wo"], f32)
    boe = (np.asarray(inputs["bo"], f32)
           + Wo @ np.asarray(inputs["bv"], f32))

    alpha = np.asarray(inputs["alpha"], f32)
    beta = np.asarray(inputs["beta"], f32)
    alphap = (alpha / s1).astype(f32)
    rbp = (s1 / (beta + 1e-9)).astype(f32)
    c2 = np.full((P, 1), 1.0 / (s1 * s2), f32)

    common = {
        "gsem": cols(inputs["g_sem"], M_SEM),
        "gpro": cols(inputs["g_pro"], M_PRO),
        "gff": cols(inputs["g_ff"], M_SEM),
        "bq": cols(inputs["bq"], M_SEM),
        "bk": cols(inputs["bk"], M_SEM),
        "boe": cols(boe, M_SEM),
        "alphap": cols(alphap, M_FF),
        "rbp": cols(rbp, M_FF),
        "c2": c2,
        "wq": _feat_major(np.asarray(inputs["Wq"], f32).T * WSC, M_SEM).astype(f8),
        "wk": _feat_major(np.asarray(inputs["Wk"], f32).T * WSC, M_PRO).astype(f8),
        "wv": _feat_major(np.asarray(inputs["Wv"], f32).T * WSC, M_PRO).astype(f8),
        "wo": _feat_major(Wo.T * WSC, M_SEM).astype(f8),
        "w1q": _feat_major(np.ascontiguousarray(w1t.T), M_SEM).astype(f8),
        "w2q": _feat_major(np.ascontiguousarray(w2t.T), M_FF).astype(f8),
    }

    in_maps = []
    for c in range(N_CORES):
        b, half = c // 2, c % 2
        m = dict(common)
        m["semT"] = _feat_major(
            np.ascontiguousarray(sem[b, half * TOK:(half + 1) * TOK, :].T),
            M_SEM)
        m["proT"] = _feat_major(np.ascontiguousarray(pro[b].T), M_PRO)
        in_maps.append(m)
    return in_maps


def assemble_out(results):
    out = np.empty((B, S, DS), np.float32)
    for c in range(N_CORES):
        b, half = c // 2, c % 2
        out[b, half * TOK:(half + 1) * TOK, :] = results[c]["outT"].T
    return out


def kernel(**inputs):
    nc = _get_nc()
    in_maps = make_in_maps(inputs)
    res = run_bass_kernel_spmd(nc, in_maps, core_ids=list(range(N_CORES)))
    return assemble_out(res.results)


# revision 54
# speedup vs baseline: 1.0959x; 1.0055x over previous
"""Trainium2 Bass kernel for nn_CrossAttentionFusion (cross-attention + BitLinear FFN).

Sharding: 8 cores = 4 batches x 2 sequence-halves. Each core:
  - owns 1024 query tokens (sem shard, feature-major),
  - computes K/V for its batch's full 2048 tokens from pro (feature-major),
  - runs full attention for its queries + BitLinear FFN, writes its out^T shard.
No collectives; host does layout prep, weight ternarization and the gather.

v2: fp8 DoubleRow matmuls for all GEMMs except QK^T scores; PE-based softmax
denominator (DR all-ones stationary gives a broadcast denominator directly);
act-quant implemented as a direct fp8 cast with all static scales folded into
the snake/evac constants; bf16 residual trunk; 2-chunk pipeline overlapping
scalar-bound attention with PE-bound FFN2.
"""
import math
import numpy as np
from contextlib import ExitStack

import concourse.bass as bass
import concourse.tile as tile
from concourse import bacc, mybir
from concourse.bass_utils import run_bass_kernel_spmd

F32 = mybir.dt.float32
BF16 = mybir.dt.bfloat16
FP8 = mybir.dt.float8e4
AF = mybir.ActivationFunctionType
ALU = mybir.AluOpType
DR = mybir.MatmulPerfMode.DoubleRow

B, S, DS, DP, H = 4, 2048, 1024, 512, 8
DF = 4 * DS
HD = DS // H          # 128
TOK = 1024            # query tokens per core
N_CORES = 8
EPS = 1e-6
QK_SCALE = 1.0 / math.sqrt(HD)
WSC = 64.0            # host premultiplier on Wq/Wk/Wv/Wo before fp8 cast

P = 128
M_SEM = DS // P       # 8
M_PRO = DP // P       # 4
M_FF = DF // P        # 32
NT_Q = TOK // 512     # 2
NT_K = S // P         # 16


def build_nc(debug_outs=False):
    nc = bacc.Bacc("TRN2", target_bir_lowering=False, debug=False,
                   num_devices=N_CORES)

    semT = nc.dram_tensor("semT", [P, M_SEM, TOK], F32, kind="ExternalInput").ap()
    proT = nc.dram_tensor("proT", [P, M_PRO, S], F32, kind="ExternalInput").ap()
    wq_d = nc.dram_tensor("wq", [P, M_SEM, DS], FP8, kind="ExternalInput").ap()
    wk_d = nc.dram_tensor("wk", [P, M_PRO, DS], FP8, kind="ExternalInput").ap()
    wv_d = nc.dram_tensor("wv", [P, M_PRO, DS], FP8, kind="ExternalInput").ap()
    wo_d = nc.dram_tensor("wo", [P, M_SEM, DS], FP8, kind="ExternalInput").ap()
    w1_d = nc.dram_tensor("w1q", [P, M_SEM, DF], FP8, kind="ExternalInput").ap()
    w2_d = nc.dram_tensor("w2q", [P, M_FF, DS], FP8, kind="ExternalInput").ap()
    gsem = nc.dram_tensor("gsem", [P, M_SEM], F32, kind="ExternalInput").ap()
    gpro = nc.dram_tensor("gpro", [P, M_PRO], F32, kind="ExternalInput").ap()
    gff = nc.dram_tensor("gff", [P, M_SEM], F32, kind="ExternalInput").ap()
    bq_d = nc.dram_tensor("bq", [P, M_SEM], F32, kind="ExternalInput").ap()
    bk_d = nc.dram_tensor("bk", [P, M_SEM], F32, kind="ExternalInput").ap()
    boe_d = nc.dram_tensor("boe", [P, M_SEM], F32, kind="ExternalInput").ap()
    alp_d = nc.dram_tensor("alphap", [P, M_FF], F32, kind="ExternalInput").ap()
    rbp_d = nc.dram_tensor("rbp", [P, M_FF], F32, kind="ExternalInput").ap()
    c2_d = nc.dram_tensor("c2", [P, 1], F32, kind="ExternalInput").ap()
    outT = nc.dram_tensor("outT", [DS, TOK], F32, kind="ExternalOutput").ap()

    dbg = {}
    if debug_outs:
        for name, shape, dt in [
            ("dbg_semn", [P, M_SEM, TOK], FP8), ("dbg_q", [P, M_SEM, TOK], FP8),
            ("dbg_k", [P, M_SEM, S], FP8), ("dbg_v", [P, NT_K, DS], FP8),
            ("dbg_ctx", [P, M_SEM, TOK], FP8),
            ("dbg_semout", [P, M_SEM, TOK], BF16),
            ("dbg_xq", [P, M_SEM, TOK], FP8), ("dbg_hq", [P, M_FF, TOK], FP8),
        ]:
            dbg[name] = nc.dram_tensor(name, shape, dt, kind="ExternalOutput").ap()

    with tile.TileContext(nc) as tc, ExitStack() as top:
        persist = top.enter_context(tc.tile_pool(name="persist", bufs=1))
        # PSUM: 2x2 banks for scores + 2x2 banks for everything else
        ps_big = top.enter_context(tc.tile_pool(name="ps_big", bufs=2, space="PSUM"))
        ps_mm = top.enter_context(tc.tile_pool(name="ps_mm", bufs=2, space="PSUM"))

        # ---- constants ----
        ones_bf = persist.tile([P, 1], BF16)
        nc.vector.memset(ones_bf[:], 1.0)
        ones_f32 = persist.tile([1, P], F32)
        nc.vector.memset(ones_f32[:], 1.0)
        ones_dr = persist.tile([P, 2, P], FP8)
        nc.vector.memset(ones_dr[:].rearrange("p a b -> p (a b)"), 1.0)
        eps_t = persist.tile([P, 1], F32)
        nc.vector.memset(eps_t[:], EPS)

        gsem_sb = persist.tile([P, M_SEM], F32)
        gpro_sb = persist.tile([P, M_PRO], F32)
        gff_sb = persist.tile([P, M_SEM], F32)
        bq_sb = persist.tile([P, M_SEM], F32)
        bk_sb = persist.tile([P, M_SEM], F32)
        boe_sb = persist.tile([P, M_SEM], F32)
        alp_sb = persist.tile([P, M_FF], F32)
        rbp_sb = persist.tile([P, M_FF], F32)
        c2_sb = persist.tile([P, 1], F32)
        for ap_d, t in [(gsem, gsem_sb), (gpro, gpro_sb), (gff, gff_sb),
                        (bq_d, bq_sb), (bk_d, bk_sb), (boe_d, boe_sb),
                        (alp_d, alp_sb), (rbp_d, rbp_sb), (c2_d, c2_sb)]:
            nc.sync.dma_start(t[:], ap_d[:])

        # ---- big weights, resident; DMA'd later on the gpsimd queue ----
        wf = top.enter_context(tc.tile_pool(name="wf", bufs=1, side="right"))
        w1_sb = wf.tile([P, M_SEM, DF], FP8)
        w2_sb = wf.tile([P, M_FF, DS], FP8)

        def rmsnorm(pool, xs, nm, T, g_sb, out_fp8, Dtot, sq_vec=False):
            """feature-major rmsnorm -> fp8. xs(m) -> [P, T] f32/bf16 tile.
            Fully chunk-pipelined: each 512-column chunk runs square -> sum ->
            ln -> exp -> broadcast -> scale independently."""
            for ch in range(T // 512):
                ps = ps_mm.tile([P, 1024], F32, tag="mm")
                for m in range(nm):
                    sq = pool.tile([P, 512], BF16, tag="nsq", bufs=3)
                    xsl = xs(m)[:, ch * 512:(ch + 1) * 512]
                    if sq_vec:
                        nc.vector.tensor_tensor(sq[:], xsl, xsl, op=ALU.mult)
                    else:
                        nc.scalar.activation(sq[:], xsl, AF.Square)
                    nc.tensor.matmul(ps[0:1, 0:512], ones_bf[:], sq[:],
                                     start=(m == 0), stop=(m == nm - 1))
                rs_row = pool.tile([1, 512], F32, tag="rsrow", bufs=2)
                nc.scalar.activation(rs_row[:], ps[0:1, 0:512], AF.Ln,
                                     bias=eps_t[0:1, :], scale=1.0 / Dtot)
                nc.scalar.activation(rs_row[:], rs_row[:], AF.Exp, scale=-0.5)
                psb = ps_mm.tile([P, 1024], F32, tag="mm")
                nc.tensor.matmul(psb[:, 0:512], ones_f32[:], rs_row[:],
                                 start=True, stop=True)
                for m in range(nm):
                    nc.vector.scalar_tensor_tensor(
                        out=out_fp8[:, m, ch * 512:(ch + 1) * 512],
                        in0=xs(m)[:, ch * 512:(ch + 1) * 512],
                        scalar=g_sb[:, m:m + 1], in1=psb[:, 0:512],
                        op0=ALU.mult, op1=ALU.mult)

        # out-proj weights: allocated early so pool release order stays LIFO
        es_wo = ExitStack()
        pwo = es_wo.enter_context(tc.tile_pool(name="pwo", bufs=1))
        wo_sb = pwo.tile([P, M_SEM, DS], FP8)

        # QKV weights allocated before pin so their DMAs start immediately
        es_w1 = ExitStack()
        pw1 = es_w1.enter_context(tc.tile_pool(name="pw1", bufs=1))
        wq_sb = pw1.tile([P, M_SEM, DS], FP8)
        wk_sb = pw1.tile([P, M_PRO, DS], FP8)
        wv_sb = pw1.tile([P, M_PRO, DS], FP8)

        # ================= P0: input norms =================
        es_nrm = ExitStack()
        pnorm = es_nrm.enter_context(tc.tile_pool(name="pnorm", bufs=1))
        semn = pnorm.tile([P, M_SEM, TOK], FP8)
        pron = pnorm.tile([P, M_PRO, S], FP8)

        with tc.tile_pool(name="pin", bufs=1) as pin:
            semT_sb = pin.tile([P, M_SEM, TOK], F32)
            proT_sb = pin.tile([P, M_PRO, S], F32)
            # chunked so the first rmsnorm square can start after 0.5 MB;
            # spread across three DMA queues so transfers run in parallel
            for m in range(M_SEM):
                nc.sync.dma_start(semT_sb[:, m, :], semT[:, m, :])
            for m in range(M_PRO):
                nc.sync.dma_start(proT_sb[:, m, :], proT[:, m, :])
            nc.sync.dma_start(wq_sb[:], wq_d[:])
            nc.sync.dma_start(wk_sb[:], wk_d[:])
            nc.sync.dma_start(wv_sb[:], wv_d[:])
            nc.sync.dma_start(w1_sb[:], w1_d[:])
            nc.sync.dma_start(w2_sb[:], w2_d[:])
            nc.sync.dma_start(wo_sb[:], wo_d[:])
            rmsnorm(pin, lambda m: semT_sb[:, m, :], M_SEM, TOK, gsem_sb,
                    semn, DS)
            rmsnorm(pin, lambda m: proT_sb[:, m, :], M_PRO, S, gpro_sb,
                    pron, DP)

        if debug_outs:
            nc.sync.dma_start(dbg["dbg_semn"][:], semn[:])

        # ================= P1: Q/K/V projections =================
        # pff (FFN scratch) sits below pqkv on the right stack so q/k/v can
        # be released before the FFN tail while pff lives on
        es_ff = ExitStack()
        pff = es_ff.enter_context(tc.tile_pool(name="pff", bufs=1,
                                               side="right"))
        es_qkv = ExitStack()
        pqkv = es_qkv.enter_context(tc.tile_pool(name="pqkv", bufs=1,
                                                 side="right"))
        q_sb = pqkv.tile([P, M_SEM, TOK], FP8)
        k_sb = pqkv.tile([P, M_SEM, S], FP8)
        v_sb = pqkv.tile([P, NT_K, DS], FP8)

        # Q/K/V interleaved per head so attention can start at head 0;
        # Q/K evac on the scalar engine, V on vector (separate queues)
        for m in range(M_SEM):
            ps = (ps_mm if m % 2 else ps_big).tile(
                [P, 1024], F32, tag="mm" if m % 2 else "big")
            for n in range(NT_Q):
                for kp in range(M_SEM // 2):
                    nc.tensor.matmul(
                        ps[:, n * 512:(n + 1) * 512],
                        wq_sb[:, 2 * kp:2 * kp + 2, m * P:(m + 1) * P],
                        semn[:, 2 * kp:2 * kp + 2, n * 512:(n + 1) * 512],
                        start=(kp == 0), stop=(kp == M_SEM // 2 - 1),
                        perf_mode=DR)
            nc.scalar.activation(q_sb[:, m, :], ps[:], AF.Identity,
                                 bias=bq_sb[:, m:m + 1], scale=1.0 / WSC)
            for chp in range(2):
                ps = (ps_mm if chp else ps_big).tile(
                    [P, 1024], F32, tag="mm" if chp else "big")
                for half in range(2):
                    ch = 2 * chp + half
                    for kp in range(M_PRO // 2):
                        nc.tensor.matmul(
                            ps[:, half * 512:(half + 1) * 512],
                            wk_sb[:, 2 * kp:2 * kp + 2, m * P:(m + 1) * P],
                            pron[:, 2 * kp:2 * kp + 2,
                                 ch * 512:(ch + 1) * 512],
                            start=(kp == 0), stop=(kp == M_PRO // 2 - 1),
                            perf_mode=DR)
                nc.scalar.activation(
                    k_sb[:, m, chp * 1024:(chp + 1) * 1024], ps[:],
                    AF.Identity, bias=bk_sb[:, m:m + 1], scale=1.0 / WSC)
            for mt in (2 * m, 2 * m + 1):
                ps = (ps_mm if mt % 2 else ps_big).tile(
                    [P, 1024], F32, tag="mm" if mt % 2 else "big")
                for ch in range(2):
                    for kp in range(M_PRO // 2):
                        nc.tensor.matmul(
                            ps[:, ch * 512:(ch + 1) * 512],
                            pron[:, 2 * kp:2 * kp + 2, mt * P:(mt + 1) * P],
                            wv_sb[:, 2 * kp:2 * kp + 2,
                                  ch * 512:(ch + 1) * 512],
                            start=(kp == 0), stop=(kp == M_PRO // 2 - 1),
                            perf_mode=DR)
                nc.vector.tensor_scalar(v_sb[:, mt, :], ps[:], 1.0 / WSC,
                                        None, ALU.mult)
        es_nrm.close()
        es_w1.close()

        if debug_outs:
            nc.sync.dma_start(dbg["dbg_q"][:], q_sb[:])
            nc.sync.dma_start(dbg["dbg_k"][:], k_sb[:])
            nc.sync.dma_start(dbg["dbg_v"][:], v_sb[:])

        # ============ pipeline state tiles ============
        es_pipe = ExitStack()
        ppipe = es_pipe.enter_context(tc.tile_pool(name="ppipe", bufs=1))
        semout_n = [ppipe.tile([P, M_SEM, 512], BF16, name=f"so{n}")
                    for n in range(NT_Q)]
        xq_n = [ppipe.tile([P, M_SEM, 512], FP8, name=f"xq{n}")
                for n in range(NT_Q)]
        es_attn = ExitStack()
        pattn = es_attn.enter_context(tc.tile_pool(name="pattn", bufs=1))

        deferred = []

        def drain(k):
            for _ in range(k):
                if deferred:
                    deferred.pop(0)()

        def attn_chunk(n, ctx_t):
            """softmax(q_n K^T) V -> ctx_t [P, M_SEM, 512] fp8 (head-major)."""
            for h in range(H):
                pt = pattn.tile([P, NT_K, 512], FP8, tag="pt", bufs=2)
                for g in range(8):
                    ps = ps_big.tile([P, 1024], F32, tag="big")
                    for q2 in range(2):
                        mt = 2 * g + q2
                        nc.tensor.matmul(
                            ps[:, q2 * 512:(q2 + 1) * 512],
                            k_sb[:, h, mt * P:(mt + 1) * P],
                            q_sb[:, h, n * 512:(n + 1) * 512],
                            start=True, stop=True)
                    drain(1)
                    nc.scalar.activation(pt[:, 2 * g:2 * g + 2, :], ps[:],
                                         AF.Exp, scale=QK_SCALE)

                box = []

                def fin_den(h=h, pt=pt, box=box):
                    pdc = ps_mm.tile([P, 1024], F32, tag="mm")
                    for j in range(8):
                        nc.tensor.matmul(pdc[:, 0:512], ones_dr[:],
                                         pt[:, 2 * j:2 * j + 2, :],
                                         start=(j == 0), stop=(j == 7),
                                         perf_mode=DR)
                    rden = pattn.tile([P, 512], F32, tag="rden", bufs=2)
                    nc.vector.reciprocal_approx_fast(rden[:], pdc[:, 0:512])
                    box.append((pdc, rden))

                def fin_ctx(h=h, pt=pt, box=box):
                    pdc, rden = box.pop()
                    for j in range(8):
                        nc.tensor.matmul(pdc[:, 512:1024],
                                         v_sb[:, 2 * j:2 * j + 2,
                                              h * P:(h + 1) * P],
                                         pt[:, 2 * j:2 * j + 2, :],
                                         start=(j == 0), stop=(j == 7),
                                         perf_mode=DR)
                    nc.vector.tensor_tensor(ctx_t[:, h, :], pdc[:, 512:1024],
                                            rden[:], op=ALU.mult)
                deferred.append(fin_den)
                deferred.append(fin_ctx)
            drain(len(deferred))

        def out_proj(n, ctx_t, mgs=None):
            for mg in (range(M_SEM // 2) if mgs is None else mgs):
                semres = ppipe.tile([P, 2, 512], F32, tag="semres", bufs=2)
                nc.sync.dma_start(semres[:],
                                  semT[:, 2 * mg:2 * mg + 2,
                                       n * 512:(n + 1) * 512])
                ps = ps_mm.tile([P, 1024], F32, tag="mm")
                for half in range(2):
                    m = 2 * mg + half
                    for kp in range(M_SEM // 2):
                        nc.tensor.matmul(
                            ps[:, half * 512:(half + 1) * 512],
                            wo_sb[:, 2 * kp:2 * kp + 2, m * P:(m + 1) * P],
                            ctx_t[:, 2 * kp:2 * kp + 2, :],
                            start=(kp == 0), stop=(kp == M_SEM // 2 - 1),
                            perf_mode=DR)
                for half in range(2):
                    m = 2 * mg + half
                    t = pff.tile([P, 512], BF16, tag="oproj", bufs=3)
                    nc.vector.tensor_scalar(t[:],
                                            ps[:, half * 512:(half + 1) * 512],
                                            1.0 / WSC, boe_sb[:, m:m + 1],
                                            ALU.mult, ALU.add)
                    nc.vector.tensor_tensor(semout_n[n][:, m, :], t[:],
                                            semres[:, half, :], op=ALU.add)

        def ff_norm(n):
            rmsnorm(pff, lambda m: semout_n[n][:, m, :], M_SEM, 512, gff_sb,
                    xq_n[n], DS, sq_vec=True)

        def ffn1(n, hq_t, zip_fn=()):
            zip_fn = list(zip_fn)
            for mg in range(M_FF // 2):
                if mg % 4 == 1 and zip_fn:
                    zip_fn.pop(0)()
                ps = (ps_mm if mg % 2 else ps_big).tile(
                    [P, 1024], F32, tag="mm" if mg % 2 else "big")
                for half in range(2):
                    m = 2 * mg + half
                    for kp in range(M_SEM // 2):
                        nc.tensor.matmul(
                            ps[:, half * 512:(half + 1) * 512],
                            w1_sb[:, 2 * kp:2 * kp + 2, m * P:(m + 1) * P],
                            xq_n[n][:, 2 * kp:2 * kp + 2, :],
                            start=(kp == 0), stop=(kp == M_SEM // 2 - 1),
                            perf_mode=DR)
                sn = pff.tile([P, 1024], BF16, tag="sn", bufs=2)
                for half in range(2):
                    m = 2 * mg + half
                    nc.scalar.activation(sn[:, half * 512:(half + 1) * 512],
                                         ps[:, half * 512:(half + 1) * 512],
                                         AF.Sin, scale=alp_sb[:, m:m + 1])
                sq = pff.tile([P, 1024], BF16, tag="sqf", bufs=2)
                nc.vector.tensor_tensor(sq[:], sn[:], sn[:], op=ALU.mult)
                for half in range(2):
                    m = 2 * mg + half
                    nc.vector.scalar_tensor_tensor(
                        out=hq_t[:, m, :],
                        in0=sq[:, half * 512:(half + 1) * 512],
                        scalar=rbp_sb[:, m:m + 1],
                        in1=ps[:, half * 512:(half + 1) * 512],
                        op0=ALU.mult, op1=ALU.add)

        def ffn2_tile(n, mg, hq_t):
            ps = ps_mm.tile([P, 1024], F32, tag="mm")
            for half in range(2):
                m = 2 * mg + half
                for kp in range(M_FF // 2):
                    nc.tensor.matmul(
                        ps[:, half * 512:(half + 1) * 512],
                        w2_sb[:, 2 * kp:2 * kp + 2, m * P:(m + 1) * P],
                        hq_t[:, 2 * kp:2 * kp + 2, :],
                        start=(kp == 0), stop=(kp == M_FF // 2 - 1),
                        perf_mode=DR)
            for half in range(2):
                m = 2 * mg + half
                yo = pff.tile([P, 512], F32, tag="yo", bufs=2)
                nc.vector.scalar_tensor_tensor(
                    out=yo[:], in0=ps[:, half * 512:(half + 1) * 512],
                    scalar=c2_sb[:, 0:1], in1=semout_n[n][:, m, :],
                    op0=ALU.mult, op1=ALU.add)
                nc.sync.dma_start(outT[m * P:(m + 1) * P,
                                       n * 512:(n + 1) * 512], yo[:])

        # ================= P2..P6: pipeline =================
        # attn(0) -> attn(1) zipped with [out_proj(0), ff_norm(0)] ->
        # out_proj(1), ff_norm(1) -> FFN1(0) -> FFN1(1) zipped with FFN2(0)
        # -> FFN2(1).  All Sin activations are grouped at the tail so the
        # scalar engine switches act tables once (exp/ln family -> sin).
        ctx0 = pattn.tile([P, M_SEM, 512], FP8, tag="ctxt", bufs=1, name="ctx0")
        attn_chunk(0, ctx0)

        for mg in range(M_SEM // 2):
            deferred.append(lambda mg=mg: out_proj(0, ctx0, mgs=[mg]))
        deferred.append(lambda: ff_norm(0))
        ctx1 = pattn.tile([P, M_SEM, 512], FP8, tag="ctxt", bufs=1, name="ctx1")
        attn_chunk(1, ctx1)
        out_proj(1, ctx1)
        ff_norm(1)
        if debug_outs:
            nc.sync.dma_start(dbg["dbg_ctx"][:, :, 0:512], ctx0[:])
            nc.sync.dma_start(dbg["dbg_semout"][:, :, 0:512], semout_n[0][:])
            nc.sync.dma_start(dbg["dbg_xq"][:, :, 0:512], xq_n[0][:])
            nc.sync.dma_start(dbg["dbg_ctx"][:, :, 512:1024], ctx1[:])
            nc.sync.dma_start(dbg["dbg_semout"][:, :, 512:1024], semout_n[1][:])
            nc.sync.dma_start(dbg["dbg_xq"][:, :, 512:1024], xq_n[1][:])

        es_attn.close()   # pt/rden/ctx freed
        es_qkv.close()    # q/k/v freed before the FFN tail
        es_hq = ExitStack()
        phq = es_hq.enter_context(tc.tile_pool(name="phq", bufs=2,
                                               side="right"))

        hq0 = phq.tile([P, M_FF, 512], FP8, tag="hq", bufs=2, name="hq0")
        ffn1(0, hq0)
        if debug_outs:
            nc.sync.dma_start(dbg["dbg_hq"][:, :, 0:512], hq0[:])

        hq1 = phq.tile([P, M_FF, 512], FP8, tag="hq", bufs=2, name="hq1")
        ffn1(1, hq1, zip_fn=[lambda mg=mg: ffn2_tile(0, mg, hq0)
                             for mg in range(M_SEM // 2)])
        if debug_outs:
            nc.sync.dma_start(dbg["dbg_hq"][:, :, 512:1024], hq1[:])
        for mg in range(M_SEM // 2):
            ffn2_tile(1, mg, hq1)

        es_hq.close()
        es_pipe.close()
        es_ff.close()
        es_wo.close()

    nc.compile()
    return nc


_NC_CACHE = {}


def _get_nc(debug_outs=False):
    key = bool(debug_outs)
    if key not in _NC_CACHE:
        _NC_CACHE[key] = build_nc(debug_outs)
    return _NC_CACHE[key]


def _feat_major(x, nm):
    """[rows, cols] -> [128, nm, cols] with rows = nm*128 split (m p) -> p m."""
    rows, cols = x.shape
    return np.ascontiguousarray(
        x.reshape(nm, P, cols).transpose(1, 0, 2))


def make_in_maps(inputs):
    """Host-side shard + layout prep. inputs: dict of full np arrays."""
    import ml_dtypes
    f8 = ml_dtypes.float8_e4m3fn
    f32 = np.float32
    sem = np.asarray(inputs["sem"], f32)
    pro = np.asarray(inputs["pro"], f32)

    def cols(v, nm):
        return np.ascontiguousarray(np.asarray(v, f32).reshape(nm, P).T)

    W1 = np.asarray(inputs["W1"], f32)
    W2 = np.asarray(inputs["W2"], f32)
    s1 = 1.0 / max(np.abs(W1).mean(dtype=np.float64), 1e-5)
    s2 = 1.0 / max(np.abs(W2).mean(dtype=np.float64), 1e-5)
    w1t = np.clip(np.round(W1 * s1), -1, 1).astype(f32)   # [DF, DS] ternary
    w2t = np.clip(np.round(W2 * s2), -1, 1).astype(f32)   # [DS, DF] ternary

    Wo = np.asarray(inputs["Wo"], f32)
    boe = (np.asarray(inputs["bo"], f32)
           + Wo @ np.asarray(inputs["bv"], f32))

    alpha = np.asarray(inputs["alpha"], f32)
    beta = np.asarray(inputs["beta"], f32)
    alphap = (alpha / s1).astype(f32)
    rbp = (s1 / (beta + 1e-9)).astype(f32)
    c2 = np.full((P, 1), 1.0 / (s1 * s2), f32)

    common = {
        "gsem": cols(inputs["g_sem"], M_SEM),
        "gpro": cols(inputs["g_pro"], M_PRO),
        "gff": cols(inputs["g_ff"], M_SEM),
        "bq": cols(inputs["bq"], M_SEM),
        "bk": cols(inputs["bk"], M_SEM),
        "boe": cols(boe, M_SEM),
        "alphap": cols(alphap, M_FF),
        "rbp": cols(rbp, M_FF),
        "c2": c2,
        "wq": _feat_major(np.asarray(inputs["Wq"], f32).T * WSC, M_SEM).astype(f8),
        "wk": _feat_major(np.asarray(inputs["Wk"], f32).T * WSC, M_PRO).astype(f8),
        "wv": _feat_major(np.asarray(inputs["Wv"], f32).T * WSC, M_PRO).astype(f8),
        "wo": _feat_major(Wo.T * WSC, M_SEM).astype(f8),
        "w1q": _feat_major(np.ascontiguousarray(w1t.T), M_SEM).astype(f8),
        "w2q": _feat_major(np.ascontiguousarray(w2t.T), M_FF).astype(f8),
    }

    in_maps = []
    for c in range(N_CORES):
        b, half = c // 2, c % 2
        m = dict(common)
        m["semT"] = _feat_major(
            np.ascontiguousarray(sem[b, half * TOK:(half + 1) * TOK, :].T),
            M_SEM)
        m["proT"] = _feat_major(np.ascontiguousarray(pro[b].T), M_PRO)
        in_maps.append(m)
    return in_maps


def assemble_out(results):
    out = np.empty((B, S, DS), np.float32)
    for c in range(N_CORES):
        b, half = c // 2, c % 2
        out[b, half * TOK:(half + 1) * TOK, :] = results[c]["outT"].T
    return out


def kernel(**inputs):
    nc = _get_nc()
    in_maps = make_in_maps(inputs)
    res = run_bass_kernel_spmd(nc, in_maps, core_ids=list(range(N_CORES)))
    return assemble_out(res.results)
